# revision 20
# baseline (speedup 1.0000x reference)
"""Trainium2 Bass kernel for nn_BinReg (histogram_binning dampening loss).

Computes: 0.1 * ( mean((wq - w)^2) + sum_k var_k ) where var_k is the
unbiased variance of w restricted to quant-bin k (16 bins), added only
when count_k > 1.

Strategy (8 NeuronCores, data-parallel over elements):
  - Shard the 4096x16384 tensors row-wise into 8 shards of [512, 16384],
    viewed as [128 partitions, 65536 free] per core.
  - Per core, stream tiles [128, FT].  Bin ids b = wq/alpha + 8 + 192 in
    bf16 land exactly on 192+k (bf16 ULP=1 in [128,256)), so is_equal
    masks are exact.  Accumulating ops run at 1x on HW (~4us per
    [128,4096] pass on any engine), so the layout is a straight
    DVE/ACT pass-count balance with GPSIMD doing the b_bf prep:
      GPSIMD: b_bf = wq/alpha + 200 (bf16 affine)
      DVE:    s_k for bins 0..14 via fused STT (b==k)*w with
              free-dim accumulate (also materializes mw_k);
              cnt_k for bins 0..CNT_DVE-1 via is_equal tensor_scalar
              accumulate; tots = sum(w).
      ACT:    ss_k for bins 0..14 via Square(mw_k) accumulate;
              totss = sum(w^2); cnt_k for the remaining bins via
              relu-difference: R_t = sum relu(b - t) for
              t = CNT_DVE-1..14, cnt_k = R_{k-1} - 2 R_k + R_{k+1}
              (per-partition bias tiles; R_15 = 0).
    Bin 15 falls out by subtraction from tots/totss on the host; its
    count is positional.
  - MSE is reconstructed on the host from the bin stats:
      sum((wq-w)^2) = a^2*sum_k cnt_k q_k^2 - 2a*sum_k q_k s_k + sum_k ss_k
    (wq sits exactly on the quant grid by construction; verified to
    9e-9 relative against the direct sum).
  - Per-core partial sums land in SBUF accumulator columns (one column
    per (bin, tile)); DMA'd out and reduced on the host in float64.
"""

from functools import lru_cache

import numpy as np

import concourse.bacc as bacc
import concourse.bass as bass
import concourse.mybir as mybir
import concourse.tile as tile
from concourse.bass_utils import run_bass_kernel_spmd

P = 128
N_CORES = 8
ROWS, COLS = 4096, 16384
SHARD_ROWS = ROWS // N_CORES            # 512
FREE = SHARD_ROWS * COLS // P           # 65536 elements per partition
FT = 4096                               # tile free size
NBINS = 16
NB = NBINS - 1                          # bins computed on-device

F32 = mybir.dt.float32
BF16 = mybir.dt.bfloat16
ALU = mybir.AluOpType
ACTF = mybir.ActivationFunctionType
BMAG = 192.0  # bf16 magic base: b lands exactly on 192+k (ULP=1 in [128,256))

# Set by test.py; results stashed for inspection.
TRACE = False
LAST_RESULTS = None
REPEAT = 1      # timing aid: repeat the whole compute R times (same result)
CNT_DVE = 7     # bins whose count runs on DVE; the rest via ACT relu-diff
PREP_ENG = "gpsimd"  # engine for the b_bf prep pass: gpsimd|dve|act


@lru_cache(maxsize=32)
def _build(inv_alpha: float, free: int = FREE, ft: int = FT,
           repeat: int = 1, cnt_dve: int = CNT_DVE,
           prep_eng: str = PREP_ENG) -> bass.Bass:
    NT = free // ft
    # relu thresholds t = cnt_dve .. 14; R_t = sum_{k>t} cnt_k*(k-t) is
    # triangular in cnt_{cnt_dve+1..15}, and cnt_{cnt_dve} follows from the
    # (positional) total count, so the system is full rank.
    ts = list(range(cnt_dve, NB))
    nR = len(ts)
    nc = bacc.Bacc(trn_type="TRN2")
    w_d = nc.dram_tensor("w", [P, free], F32, kind="ExternalInput")
    wq_d = nc.dram_tensor("wq", [P, free], F32, kind="ExternalInput")
    cnt_d = nc.dram_tensor("cnt", [P, max(cnt_dve, 1) * NT], F32,
                           kind="ExternalOutput")
    rr_d = nc.dram_tensor("rr", [P, max(nR, 1) * NT], F32,
                          kind="ExternalOutput")
    s_d = nc.dram_tensor("s", [P, NB * NT], F32, kind="ExternalOutput")
    ss_d = nc.dram_tensor("ss", [P, NB * NT], F32, kind="ExternalOutput")
    tots_d = nc.dram_tensor("tots", [P, NT], F32, kind="ExternalOutput")
    totss_d = nc.dram_tensor("totss", [P, NT], F32, kind="ExternalOutput")

    with tile.TileContext(nc) as tc:
        with (
            tc.tile_pool(name="io", bufs=2) as io,
            tc.tile_pool(name="work", bufs=2) as work,
            tc.tile_pool(name="acc", bufs=1) as acc,
        ):
            cnt_a = acc.tile([P, max(cnt_dve, 1) * NT], F32, tag="cnt_a")
            rr_a = acc.tile([P, max(nR, 1) * NT], F32, tag="rr_a")
            s_a = acc.tile([P, NB * NT], F32, tag="s_a")
            ss_a = acc.tile([P, NB * NT], F32, tag="ss_a")
            tots_a = acc.tile([P, NT], F32, tag="tots_a")
            totss_a = acc.tile([P, NT], F32, tag="totss_a")
            if cnt_dve == 0:
                nc.gpsimd.memset(cnt_a[:], 0.0)
            if nR == 0:
                nc.gpsimd.memset(rr_a[:], 0.0)
            bias_t = acc.tile([P, max(nR, 1)], F32, tag="bias_t")
            for j, t in enumerate(ts):
                nc.gpsimd.memset(bias_t[:, j : j + 1], -(BMAG + float(t)))

            import contextlib
            loop_cm = (
                tc.For_i(
                    0, repeat, 1,
                    hint_engines=(
                        mybir.EngineType.DVE,
                        mybir.EngineType.Activation,
                        mybir.EngineType.Pool,
                    ),
                )
                if repeat > 1
                else contextlib.nullcontext()
            )
            with loop_cm:
                for i in range(NT):
                    sl = slice(i * ft, (i + 1) * ft)
                    w_t = io.tile([P, ft], F32, tag="w")
                    nc.sync.dma_start(w_t[:], w_d[:, sl])
                    wq_t = io.tile([P, ft], F32, tag="wq")
                    nc.sync.dma_start(wq_t[:], wq_d[:, sl])

                    # b = wq/alpha + 8 + 192, bf16-rounds exactly to 192+k
                    b_bf = work.tile([P, ft], BF16, tag="b_bf")
                    if prep_eng == "gpsimd":
                        nc.gpsimd.tensor_scalar(
                            b_bf[:], wq_t[:], inv_alpha, BMAG + 8.0,
                            op0=ALU.mult, op1=ALU.add,
                        )
                    elif prep_eng == "act":
                        nc.scalar.activation(
                            b_bf[:], wq_t[:], ACTF.Copy,
                            bias=BMAG + 8.0, scale=inv_alpha,
                        )
                    else:
                        nc.vector.tensor_scalar(
                            b_bf[:], wq_t[:], inv_alpha, BMAG + 8.0,
                            op0=ALU.mult, op1=ALU.add,
                        )

                    # totals for bin-15-by-subtraction
                    tj_t = work.tile([P, ft], BF16, tag="junk_dve")
                    nc.vector.tensor_scalar(
                        tj_t[:], w_t[:], 1.0, None,
                        op0=ALU.mult, op1=ALU.add,
                        accum_out=tots_a[:, i : i + 1],
                    )
                    tsq_t = work.tile([P, ft], BF16, tag="junk_act")
                    nc.scalar.activation(
                        tsq_t[:], w_t[:], ACTF.Square,
                        accum_out=totss_a[:, i : i + 1],
                    )

                    # R_t = sum relu(b - t) on ACT (per-partition bias)
                    for j, t in enumerate(ts):
                        rj_t = work.tile([P, ft], BF16, tag="junk_act")
                        nc.scalar.activation(
                            rj_t[:], b_bf[:], ACTF.Relu,
                            bias=bias_t[:, j : j + 1],
                            accum_out=rr_a[:, j * NT + i : j * NT + i + 1],
                        )

                    for k in range(NB):
                        col = k * NT + i
                        # masked w + fused sum -> s_k (one STT on DVE)
                        mw_t = work.tile([P, ft], BF16, tag=f"mw{k % 3}")
                        nc.vector.scalar_tensor_tensor(
                            mw_t[:], b_bf[:], BMAG + float(k), w_t[:],
                            op0=ALU.is_equal, op1=ALU.mult,
                            accum_out=s_a[:, col : col + 1],
                        )
                        # ss_k on ACT
                        sq_t = work.tile([P, ft], BF16, tag="junk_act")
                        nc.scalar.activation(
                            sq_t[:], mw_t[:], ACTF.Square,
                            accum_out=ss_a[:, col : col + 1],
                        )
                        # cnt_k: DVE is_equal accumulate for low bins
                        if k < cnt_dve:
                            c_t = work.tile([P, ft], BF16, tag="junk_dve")
                            nc.vector.tensor_scalar(
                                c_t[:], b_bf[:], BMAG + float(k), None,
                                op0=ALU.is_equal, op1=ALU.add,
                                accum_out=cnt_a[:, k * NT + i : k * NT + i + 1],
                            )

            nc.sync.dma_start(cnt_d[:], cnt_a[:])
            nc.sync.dma_start(rr_d[:], rr_a[:])
            nc.sync.dma_start(s_d[:], s_a[:])
            nc.sync.dma_start(ss_d[:], ss_a[:])
            nc.sync.dma_start(tots_d[:], tots_a[:])
            nc.sync.dma_start(totss_d[:], totss_a[:])

    nc.finalize()
    return nc


def _reduce_stats(results, free, cnt_dve=CNT_DVE):
    NT = free // FT
    ts = list(range(cnt_dve, NB))
    nR = len(ts)
    cnt = np.zeros(NBINS, dtype=np.float64)
    s = np.zeros(NBINS, dtype=np.float64)
    ss = np.zeros(NBINS, dtype=np.float64)
    rr = np.zeros(max(nR, 1), dtype=np.float64)
    n_total = 0.0
    for r in results:
        if cnt_dve > 0:
            cnt[:cnt_dve] += (
                r["cnt"].astype(np.float64).reshape(P, cnt_dve, NT)
                .sum(axis=(0, 2))
            )
        s[:NB] += r["s"].astype(np.float64).reshape(P, NB, NT).sum(axis=(0, 2))
        ss[:NB] += r["ss"].astype(np.float64).reshape(P, NB, NT).sum(axis=(0, 2))
        s[NB] += float(r["tots"].astype(np.float64).sum())
        ss[NB] += float(r["totss"].astype(np.float64).sum())
        if nR:
            rr += r["rr"].astype(np.float64).reshape(P, nR, NT).sum(axis=(0, 2))
        n_total += P * free
    # R_t = sum_{k>t} cnt_k*(k-t) for t = cnt_dve..14: back-substitute from
    # t=14 (R_14 = cnt_15) down, then cnt_{cnt_dve} from the global total.
    for t in range(NB - 1, cnt_dve - 1, -1):
        j = t - cnt_dve
        ks = np.arange(t + 2, NBINS)
        cnt[t + 1] = np.round(rr[j] - ((ks - t) * cnt[t + 2 :]).sum())
    cnt[cnt_dve] = n_total - cnt.sum() + cnt[cnt_dve]
    s[NB] -= s[:NB].sum()
    ss[NB] -= ss[:NB].sum()
    return cnt, s, ss, n_total


def kernel(weight, weight_q, nbit, alpha) -> np.ndarray:
    global LAST_RESULTS
    nb = int(np.asarray(nbit))
    qn = -(2 ** (nb - 1))
    qp = 2 ** (nb - 1) - 1
    nbins = qp - qn + 1
    assert nbins == NBINS, f"kernel hardcodes 16 bins, got {nbins}"
    a = float(np.asarray(alpha).reshape(-1)[0])

    w = np.ascontiguousarray(np.asarray(weight, dtype=np.float32)).reshape(
        N_CORES, P, FREE
    )
    wq = np.ascontiguousarray(np.asarray(weight_q, dtype=np.float32)).reshape(
        N_CORES, P, FREE
    )

    nc = _build(1.0 / a, FREE, FT, REPEAT, CNT_DVE, PREP_ENG)
    in_maps = [{"w": w[i], "wq": wq[i]} for i in range(N_CORES)]
    res = run_bass_kernel_spmd(
        nc, in_maps, core_ids=list(range(N_CORES)), trace=TRACE
    )
    LAST_RESULTS = res

    cnt, s, ss, n_total = _reduce_stats(res.results, FREE, CNT_DVE)

    q = np.arange(NBINS, dtype=np.float64) + qn  # quant levels / alpha
    mse_sum = a * a * (cnt * q * q).sum() - 2.0 * a * (q * s).sum() + ss.sum()
    loss = mse_sum / n_total
    denom_n = np.maximum(cnt, 1.0)
    denom_nm1 = np.maximum(cnt - 1.0, 1.0)
    var = (ss - s * s / denom_n) / denom_nm1
    loss += float(np.where(cnt > 1.0, var, 0.0).sum())
    return np.asarray(0.1 * loss, dtype=np.float32)
